# revision 22
# baseline (speedup 1.0000x reference)
"""NVFP4-style activation quantizer on 8 TRN2 NeuronCores (raw bass).

Reference semantics (per 16-element block, fp32):
    s_t  = max|x| / (6*448)                      (global, needs all-reduce)
    m_b  = max|x| over block
    inv  = 6 / (m_b / s_t)
    s_b  = fp8_e4m3_roundtrip(inv)   (the 0/inf guard is dead code for this
                                      input: inv >= 6/2688 = 2.23e-3 > 2^-10)
    out  = sign(x) * fp4_121(|x|/s_t * s_b) / s_b * s_t

Core trick: a runtime-registered custom DVE op fuses the whole fp4_121
magic-round into ONE DVE instruction per tile (6 ALU stages, 1 elem/cyc):

    y = Src0 * Src1            (x fp16 * per-block c, stride-0 bcast in1)
    p = y & 0x7F800000         (exponent bits as float = 2^e(y); s0 = +inf)
    M = max(p, 1) * 6291456    (1.5*2^22 * max(2^e,1): ulp(M) = fp4 step)
    q = (y + M) - M            (RNE to the fp4_121 grid, sign handled)

fp32 internally, so no fp16 rounding of y (sim rel_l2 7.5e-3 vs the 2e-2
gate; measured on HW bit-exact vs the numpy model of this chain).

Structure: ONE read of x (full shard cached in SBUF as fp16, 16MB),
fp16 output (host upcasts), so HBM traffic is 32MB in + 16MB out/core.
Pass A (read-bandwidth-bound, ~3.1us/tile): input DMAs on the single
SYNC queue into a 4-slot xa ring, each slot gated on BOTH consumers
(ACT fp32->fp16 convert into the xh cache; DVE fp32 per-block abs-max
-- fp32 m matters: fp16 maxes flip fp8-subnormal scale buckets and
double the error).  The ACT conv stream stays pure (the rm = 1/m
quarter recips run after conv 31, hidden in the AllReduce window);
any ACT stall would gate the DMA ring.  AllReduce of the [1,128]
per-partition maxes on the GPSIMD queue, warmed once at t=0 (a second
warmup backfires: it occupies the CC pipe exactly when the real AR
wants in).  Post-AR scalars + the f8 e4m3 roundtrip run on DVE in 5
chunks (micro first chunk eats the latency); ACT derives c16 = sb/st
(2-slot chunk ring) and rs = 1/sb.  Pass B (DVE-bound ~3.5us/tile):
custom op1 per tile (in1 = c16 stride-0 broadcast -- same speed as
dense for a 1x custom op), then o = q * nicfull as a DENSE fp16
tensor_tensor (2x mode, ~1.2us) writing over the dead xh slot;
ACT materializes nicfull (3-slot ring) from rs2 with scale=st.
GPSIMD compute is deliberately unused: its Q7 TT multiply measures
18-21us/tile and its SBUF traffic slows every other engine; queue-wise
it only runs the collectives (pool-issued input DMAs on the ring
critical path cost +160us -- measured, not modeled).  A dummy custom-op
at t=0 pre-loads the DVE uop table (first use otherwise costs ~50us
post-AR).  Engine busy: DVE ~197us, ACT ~150us, read ~100us window,
AR ~9-25us, pass B ~126us.
"""

import numpy as np

FULL_SHAPE = (4, 4096, 4096)
N_CORES = 8
P = 128
TOTAL = 4 * 4096 * 4096
L = TOTAL // (N_CORES * P)   # 65536 elements per partition per core
NBLK = L // 16               # 4096 blocks per partition

F = 2048
T = L // F                   # 32 tiles
FBLK = F // 16               # 128 blocks per tile
NQ = 4
QBLK = NBLK // NQ
TQ = T // NQ                 # 8 tiles per quarter
N_XA = 4
N_Q = 2                      # q16 ring
N_NF = 3                     # nicfull ring

MAGIC = 6291456.0            # 1.5 * 2^22

# scale-chain chunks (in blocks): micro first chunk -> eat AR latency
CHUNKS = [128, 896, 1024, 1024, 1024]
CH_START = [sum(CHUNKS[:i]) for i in range(len(CHUNKS))]
NCH = len(CHUNKS)
# chunk -> number of ACT rm-quarters that must be done first
RMQ = [1, 1, 2, 3, 4]


def chunk_of_tile(t):
    b = t * FBLK
    for c in range(NCH):
        if CH_START[c] <= b < CH_START[c] + CHUNKS[c]:
            return c
    raise ValueError(t)


def register_fp4_op():
    from concourse import dve_ops
    from concourse.dve_spec import (
        Spec, Src0, Src1, C0, C1, Bin, AluOp, maxx, One, lower, _has_src1,
    )
    from concourse.dve_uop import DveOpSpec

    NAME = "FP4_SCALE_ROUND_ANT"
    for o in dve_ops.OPS:
        if o.name == NAME:
            return o

    y = Src0 * Src1
    p = Bin(AluOp.BITWISE_AND, y, C0)
    M = maxx(p, One) * C1
    q = (y + M) - M

    def ref(in0, in1, s0, s1, imm2):
        yy = in0.astype(np.float32) * np.asarray(in1, np.float32)
        pp = (yy.view(np.int32) & np.int32(0x7F800000)).view(np.float32)
        MM = np.maximum(pp, 1.0) * np.float32(s1)
        tt = (yy + MM).astype(np.float32)
        return (tt - MM).astype(np.float32)

    spec = Spec(body=q, reference=ref)
    row = max(dve_ops._SUB_OPCODE_FOR_NAME.values()) + 1
    assert row < 0x20
    dve_ops._SUB_OPCODE_FOR_NAME[NAME] = row
    uops = lower(spec, ver="v3")
    sha = DveOpSpec(name=NAME, opcode=row, uops=uops,
                    rd1_en=_has_src1(spec)).sha("v3")
    op = dve_ops.DveOp(NAME, spec, subdim=False, uops_sha={"v3": sha})
    dve_ops.OPS.append(op)
    dve_ops.CUSTOM_DVE_SPECS[NAME] = spec
    return op


def build_nc(n_cores=N_CORES):
    from contextlib import ExitStack

    import concourse.bass as bass
    from concourse import mybir

    fp4_op = register_fp4_op()

    f32 = mybir.dt.float32
    f16 = mybir.dt.float16
    f8 = mybir.dt.float8e4

    nc = bass.Bass(num_devices=n_cores, debug=False)
    x_ext = nc.declare_dram_parameter("x", [P, L], f32, isOutput=False)
    out_ext = nc.declare_dram_parameter("out", [P, L], f16, isOutput=True)
    cc_in = nc.dram_tensor("cc_in", [1, 128], f32)
    cc_out = nc.dram_tensor("cc_out", [1, 128], f32, addr_space="Shared")
    cc_warm_in = nc.dram_tensor("cc_warm_in", [1, 128], f32)
    cc_warm_out = nc.dram_tensor("cc_warm_out", [1, 128], f32,
                                 addr_space="Shared")

    def act_reciprocal(act, out, in_):
        return act.add_instruction(
            mybir.InstActivation(
                name=act.bass.get_next_instruction_name(),
                func=mybir.ActivationFunctionType.Reciprocal,
                ins=[
                    act.lower_ap(in_),
                    mybir.ImmediateValue(dtype=f32, value=0.0),
                    mybir.ImmediateValue(dtype=f32, value=1.0),
                    mybir.ImmediateValue(dtype=f32, value=0.0),
                ],
                outs=[act.lower_ap(out)],
            )
        )

    with ExitStack() as ctx:
        def sem(name):
            return ctx.enter_context(nc.semaphore(name))

        def sbuf(name, shape, dt=f32):
            return ctx.enter_context(nc.sbuf_tensor(name, shape, dt))

        s_xa = [sem(f"s_xa{i}") for i in range(N_XA)]
        s_dve = sem("s_dve")
        s_conv = sem("s_conv")   # +1 per ACT conv (count = t+1)
        s_rm = sem("s_rm")       # +1 per ACT rm quarter (count = q+1)
        s_c16 = sem("s_c16")     # +1 per ACT c16 chunk (count = c+1)
        s_rs = sem("s_rs")       # +1 per ACT rs chunk (count = c+1)
        s_nf = sem("s_nf")       # +1 per ACT nicfull tile (count = t+1)
        s_cdma = sem("s_cdma")
        s_cc = sem("s_cc")
        s_warm = sem("s_warm")
        s_out = sem("s_out")
        s_ps = sem("s_ps")       # pool memset staging

        xh = sbuf("xh", [P, T * F], f16)            # 16MB: cache + output
        xab = sbuf("xab", [P, N_XA * F])
        xa = [xab[:, i * F:(i + 1) * F] for i in range(N_XA)]
        q16 = [sbuf(f"q16_{i}", [P, F], f16) for i in range(N_Q)]
        nf16 = [sbuf(f"nf16_{i}", [P, F], f16) for i in range(N_NF)]
        m_t = sbuf("m_t", [P, NBLK])                # blockmax -> 1/m -> s_b
        rs2 = [sbuf(f"rs2_{i}", [P, 1024], f16) for i in range(2)]
        f8_t = sbuf("f8_t", [P, 1024], f8)
        c16r = [sbuf(f"c16r_{i}", [P, 1024], f16) for i in range(2)]
        gall_t = sbuf("gall_t", [P, 128])
        mxq_t = sbuf("mxq_t", [P, NQ])
        mx_t = sbuf("mx_t", [P, 1])
        g128_t = sbuf("g128_t", [P, 1])
        st_t = sbuf("st_t", [P, 1])
        rt_t = sbuf("rt_t", [P, 1])
        k6_t = sbuf("k6_t", [P, 1])
        emask_t = sbuf("emask_t", [P, 1])

        dveA = [0] * T            # s_dve after reduce(t)
        K_mxq = [0] * NQ
        K_mx = [0]
        K_sb = [0] * NCH
        tag_q = [0] * T           # s_dve after op1(t)
        CH_LAST_TILE = [
            (CH_START[c] + CHUNKS[c]) // FBLK - 1 for c in range(NCH)]
        tag_o_dve = [0] * T       # s_dve after o(t) (dve-owned)

        def b3(ap):
            return ap.rearrange("p (b s) -> p b s", s=16)

        def qs(q):
            return slice(q * QBLK, (q + 1) * QBLK)

        def xs(t):
            return slice(t * F, (t + 1) * F)

        def cslice(c):
            return slice(CH_START[c], CH_START[c] + CHUNKS[c])

        def bcast(tens, t):
            bsl = slice(t * FBLK, (t + 1) * FBLK)
            return tens[:, bsl].unsqueeze(-1).broadcast_to([P, FBLK, 16])

        with nc.Block() as block:

            @block.vector
            def _(dve):
                cnt = 0

                def tag(ins):
                    nonlocal cnt
                    ins.then_inc(s_dve)
                    cnt += 1
                    return cnt

                k_ms = tag(dve.memset(emask_t[:], float("inf")))
                # custom-op warmup: first use pays a large one-time cost;
                # run a tiny dummy now so it lands off the critical path
                dve.wait_ge(s_dve, k_ms)
                tag(dve._custom_dve(
                    fp4_op,
                    out=q16[0][:, 0:16].rearrange("p (b s) -> p b s", s=16),
                    in0=xh[:, 0:16].rearrange("p (b s) -> p b s", s=16),
                    in1=c16r[0][:, 0:1].unsqueeze(-1).broadcast_to([P, 1, 16]),
                    s0=emask_t[:],
                    s1=MAGIC,
                ))

                # ---- pass A: per-block abs max (fp32, from the xa ring;
                # 4 slots keep the dual consumer gate under the BW floor) ----
                for t in range(T):
                    dve.wait_ge(s_xa[t % N_XA], 16 * (t // N_XA + 1))
                    dveA[t] = tag(dve.tensor_reduce(
                        out=m_t[:, t * FBLK:(t + 1) * FBLK],
                        in_=b3(xa[t % N_XA]),
                        axis=mybir.AxisListType.X,
                        op=mybir.AluOpType.max,
                        apply_absolute_value=True,
                    ))
                    if (t + 1) % TQ == 0:
                        q = t // TQ
                        dve.wait_ge(s_dve, dveA[t])
                        K_mxq[q] = tag(dve.tensor_reduce(
                            out=mxq_t[:, q:q + 1], in_=m_t[:, qs(q)],
                            axis=mybir.AxisListType.X,
                            op=mybir.AluOpType.max,
                        ))
                dve.wait_ge(s_dve, K_mxq[NQ - 1])
                K_mx[0] = tag(dve.tensor_reduce(
                    out=mx_t[:], in_=mxq_t[:], axis=mybir.AxisListType.X,
                    op=mybir.AluOpType.max,
                ))

                # ---- global scalars (post-AllReduce) ----
                dve.wait_ge(s_cdma, 32)
                k_g = tag(dve.tensor_reduce(
                    out=g128_t[:], in_=gall_t[:], axis=mybir.AxisListType.X,
                    op=mybir.AluOpType.max))
                dve.wait_ge(s_dve, k_g)
                k_st = tag(dve.tensor_scalar(
                    st_t[:], g128_t[:], 1.0 / 2688.0, None,
                    op0=mybir.AluOpType.mult))
                dve.wait_ge(s_dve, k_st)
                tag(dve.tensor_scalar(
                    k6_t[:], st_t[:], 6.0, None, op0=mybir.AluOpType.mult))
                k_rt = tag(dve.reciprocal(rt_t[:], st_t[:]))

                # ---- per-block scale chain: f8 roundtrip per chunk ----
                # (rm = 1/m already in m_t from ACT, per quarter)
                def emit_chunk(c):
                    cs = cslice(c)
                    n = CHUNKS[c]
                    dve.wait_ge(s_rm, RMQ[c])
                    dve.wait_ge(s_dve, k_rt if c == 0 else K_sb[c - 1])
                    k_f8 = tag(dve.tensor_scalar(
                        f8_t[:, 0:n], m_t[:, cs], k6_t[:], None,
                        op0=mybir.AluOpType.mult))
                    dve.wait_ge(s_dve, k_f8)
                    K_sb[c] = tag(dve.tensor_copy(m_t[:, cs], f8_t[:, 0:n]))

                emit_chunk(0)
                emit_chunk(1)

                # ---- pass B: op1 (custom, bcast in1) + o (dense, 2x) ----
                next_chunk = 2
                for t in range(T):
                    # early chunks: all f8/sb done by tile 5
                    while (next_chunk < NCH
                           and t >= 2 * (next_chunk - 2) + 1):
                        emit_chunk(next_chunk)
                        next_chunk += 1
                    c = chunk_of_tile(t)
                    lo = t * FBLK - CH_START[c]
                    dve.wait_ge(s_c16, c + 1)
                    tag_q[t] = tag(dve._custom_dve(
                        fp4_op,
                        out=b3(q16[t % N_Q][:]),
                        in0=b3(xh[:, xs(t)]),
                        in1=c16r[c % 2][:, lo:lo + FBLK].unsqueeze(-1)
                            .broadcast_to([P, FBLK, 16]),
                        s0=emask_t[:],
                        s1=MAGIC,
                    ))
                    dve.wait_ge(s_nf, t + 1)
                    dve.wait_ge(s_dve, tag_q[t])
                    tag_o_dve[t] = tag(dve.tensor_tensor(
                        xh[:, xs(t)], q16[t % N_Q][:], nf16[t % N_NF][:],
                        op=mybir.AluOpType.mult))

            @block.scalar
            def _(act):
                # pass A: fp32 -> fp16 conversions into the cache (pure
                # stream: any stall here would gate the DMA ring); the rm
                # recips run after the last conv, hidden in the AR window
                for t in range(T):
                    act.wait_ge(s_xa[t % N_XA], 16 * (t // N_XA + 1))
                    act.activation(
                        xh[:, xs(t)], xa[t % N_XA],
                        mybir.ActivationFunctionType.Copy,
                    ).then_inc(s_conv)
                for q in range(NQ):
                    act.wait_ge(s_dve, K_mxq[q])
                    act_reciprocal(
                        act, m_t[:, qs(q)], m_t[:, qs(q)]).then_inc(s_rm)

                # post-AR scale chain per chunk: c16 = rt*sb -> fp16,
                # rs = 1/sb; then nicfull per tile straight from the rs
                # chunk (dense bcast materialization, scaled by st, so the
                # o-multiply runs 2x)
                def chain(c):
                    cs = cslice(c)
                    n = CHUNKS[c]
                    act.wait_ge(s_dve, K_sb[c])
                    if c >= 2:
                        act.wait_ge(s_dve, tag_q[CH_LAST_TILE[c - 2]])
                    act.activation(
                        c16r[c % 2][:, 0:n], m_t[:, cs],
                        mybir.ActivationFunctionType.Copy,
                        scale=rt_t[:],
                    ).then_inc(s_c16)
                    act_reciprocal(
                        act, rs2[c % 2][:, 0:n], m_t[:, cs]).then_inc(s_rs)

                def nf(t):
                    c = chunk_of_tile(t)
                    lo = t * FBLK - CH_START[c]
                    src_ap = rs2[c % 2][:, lo:lo + FBLK]
                    if t >= N_NF:
                        act.wait_ge(s_dve, tag_o_dve[t - N_NF])
                    act.wait_ge(s_rs, c + 1)
                    act.activation(
                        b3(nf16[t % N_NF][:]),
                        src_ap.unsqueeze(-1).broadcast_to([P, FBLK, 16]),
                        mybir.ActivationFunctionType.Copy,
                        scale=st_t[:],
                    ).then_inc(s_nf)

                chain(0)
                chain(1)
                nf(0)
                chain(2)
                for t in range(1, 8):
                    nf(t)
                chain(3)
                for t in range(8, 16):
                    nf(t)
                chain(4)
                for t in range(16, T):
                    nf(t)

            @block.gpsimd
            def _(pool):
                pool.memset(gall_t[0:1, :], 0.0).then_inc(s_ps)
                pool.wait_ge(s_ps, 1)
                pool.dma_start(out=cc_warm_in[:, :],
                               in_=gall_t[0:1, :]).then_inc(s_warm, 16)
                pool.wait_ge(s_warm, 16)
                pool.collective_compute(
                    "AllReduce",
                    mybir.AluOpType.max,
                    replica_groups=[list(range(n_cores))],
                    ins=[cc_warm_in.ap().opt()],
                    outs=[cc_warm_out.ap().opt()],
                ).then_inc(s_cc)
                pool.wait_ge(s_dve, K_mx[0])
                pool.dma_start(out=cc_in[:, :],
                               in_=mx_t[:, :]).then_inc(s_cdma, 16)
                pool.wait_ge(s_cdma, 16)
                pool.collective_compute(
                    "AllReduce",
                    mybir.AluOpType.max,
                    replica_groups=[list(range(n_cores))],
                    ins=[cc_in.ap().opt()],
                    outs=[cc_out.ap().opt()],
                ).then_inc(s_cc)

            @block.sync
            def _(sync):
                # pass A input DMAs (single queue; slot gated on consumers)
                for t in range(T):
                    if t >= N_XA:
                        sync.wait_ge(s_conv, t - N_XA + 1)
                        sync.wait_ge(s_dve, dveA[t - N_XA])
                    sync.dma_start(
                        out=xa[t % N_XA],
                        in_=x_ext[:, xs(t)],
                    ).then_inc(s_xa[t % N_XA], 16)
                # collective staging (cc_in is DMA'd by the pool queue)
                sync.wait_ge(s_cc, 2)
                sync.dma_start(
                    out=gall_t[:, :],
                    in_=cc_out.ap().broadcast_to([P, 128]),
                ).then_inc(s_cdma, 16)
                # pass B: output DMAs (fp16, from the dead xh slot)
                for t in range(T):
                    sync.wait_ge(s_dve, tag_o_dve[t])
                    sync.dma_start(
                        out=out_ext[:, xs(t)],
                        in_=xh[:, xs(t)],
                    ).then_inc(s_out, 16)
                sync.wait_ge(s_out, 16 * T)

    mybir.codegen_inst_isa_subclasses(nc)
    return nc


_CACHE = {}


def _get_nc():
    if "nc" not in _CACHE:
        _CACHE["nc"] = build_nc()
    return _CACHE["nc"]


def kernel(x: np.ndarray) -> np.ndarray:
    from concourse.bass_utils import run_bass_kernel_spmd

    x = np.asarray(x, dtype=np.float32)
    assert x.shape == FULL_SHAPE
    shards = x.reshape(N_CORES, P, L)
    in_maps = [{"x": np.ascontiguousarray(shards[i])} for i in range(N_CORES)]
    nc = _get_nc()
    res = run_bass_kernel_spmd(nc, in_maps, core_ids=list(range(N_CORES)))
    out = np.stack([np.asarray(r["out"], dtype=np.float32)
                    for r in res.results], axis=0)
    return out.reshape(FULL_SHAPE)
